# revision 7
# baseline (speedup 1.0000x reference)
"""GCN-VAE forward pass (GCNModelVAE) on 8 Trainium2 NeuronCores.

Row-shards the 8192 nodes across 8 cores (1024 rows each). All big matmuls
run in bf16 on the PE array with fp32 PSUM accumulation; the output is
saturation-dominated (the reference's exp(log_std) overflows), so bf16
operand precision is ample.

Dataflow per core (M = 1024 local nodes, P = 128 partitions):
  0. Cast adj_s/x_s/eps_s fp32 -> bf16 in DRAM (SWDGE cast-DMA).
  A. xW1_s = x_s @ W1 via transposed tiles, AllGather -> xW1_full [8192,256].
  B. hT_s = relu(xW1_full^T @ adj_s^T) [256,1024]; adj^T tiles come from
     DMA-xbar transpose loads. AllGather hT -> h_all.
  C. hWms_full = h_full @ [Wm|Ws]  [8192,256] (replicated on every core).
  D. zmT/lsT = (hW)^T @ adj_s^T    [128,1024] each.
  E. zT = zmT + epsT * exp(lsT); AllGather zT -> z_all.
  F. out_s = sigmoid(zT_s^T @ zT_all) [1024,8192] row-block of the decoder.
"""

import numpy as np

import concourse.bacc as bacc
import concourse.mybir as mybir
import concourse.tile as tile
from concourse.bass_utils import run_bass_kernel_spmd

N = 8192
F_IN = 512
H1 = 256
H2 = 128
NC = 8
M = N // NC          # 1024 rows per core
P = 128
KT = N // P          # 64 k-tiles over the node dimension
MT = M // P          # 8 m-tiles per core
F32 = mybir.dt.float32
BF16 = mybir.dt.bfloat16
AF = mybir.ActivationFunctionType


def build(n_cores=NC):
    nc = bacc.Bacc("TRN2", target_bir_lowering=False, debug=False,
                   num_devices=n_cores)
    x_s = nc.dram_tensor("x_s", [M, F_IN], F32, kind="ExternalInput")
    adj_s = nc.dram_tensor("adj_s", [M, N], F32, kind="ExternalInput")
    W1 = nc.dram_tensor("W1", [F_IN, H1], F32, kind="ExternalInput")
    Wm = nc.dram_tensor("Wm", [H1, H2], F32, kind="ExternalInput")
    Ws = nc.dram_tensor("Ws", [H1, H2], F32, kind="ExternalInput")
    eps_s = nc.dram_tensor("eps_s", [M, H2], F32, kind="ExternalInput")
    out_s = nc.dram_tensor("out_s", [M, N], F32, kind="ExternalOutput")

    rg = [list(range(n_cores))]

    with tile.TileContext(nc) as tc:
        with (
            tc.tile_pool(name="dram", bufs=1, space="DRAM") as dram,
            tc.tile_pool(name="pers", bufs=1) as pers,
            tc.tile_pool(name="mv", bufs=6) as mv,
            tc.tile_pool(name="ev", bufs=6) as ev,
        ):
            # ---------------- DRAM staging ----------------
            adj_b = dram.tile([M, N], BF16)
            x_b = dram.tile([M, F_IN], BF16)
            eps_b = dram.tile([M, H2], BF16)
            xw1_bounce = dram.tile([M, H1], BF16)
            xw1_all = dram.tile([N, H1], BF16, addr_space="Shared")
            h_bounce = dram.tile([H1, M], BF16)
            h_all = dram.tile([H1 * NC, M], BF16, addr_space="Shared")
            z_bounce = dram.tile([H2, M], BF16)
            z_all = dram.tile([H2 * NC, M], BF16, addr_space="Shared")

            # ---------------- stage 0: fp32 -> bf16 casts ----------------
            # adj by column chunks so pass-1 k-tiles can start early.
            for c in range(NC):
                nc.gpsimd.dma_start(adj_b[:, c * M:(c + 1) * M],
                                    adj_s[:, c * M:(c + 1) * M])
            nc.gpsimd.dma_start(x_b[:, :], x_s[:, :])
            nc.gpsimd.dma_start(eps_b[:, :], eps_s[:, :])

            # W1 as moving tiles [128, 4*256]: W1b[p, kt*H1+n] = W1[kt*128+p, n]
            W1b = pers.tile([P, (F_IN // P) * H1], BF16)
            nc.gpsimd.dma_start(
                W1b[:].rearrange("p (t n) -> p t n", n=H1),
                W1.rearrange("(t p) n -> p t n", p=P))
            # [Wm | Ws] moving tiles [128, 2*256]
            Wms = pers.tile([P, (H1 // P) * (2 * H2)], BF16)
            for dt in range(H1 // P):
                nc.gpsimd.dma_start(Wms[:, dt * 256:dt * 256 + H2],
                                    Wm[dt * P:(dt + 1) * P, :])
                nc.gpsimd.dma_start(Wms[:, dt * 256 + H2:dt * 256 + 256],
                                    Ws[dt * P:(dt + 1) * P, :])

            # ---------------- stage A: xW1 shard + AllGather ----------------
            # xT tiles [128, 1024] per 128-col block of x (4 blocks)
            xT = pers.tile([P, (F_IN // P) * M], BF16)
            for kt in range(F_IN // P):
                nc.sync.dma_start(xT[:, kt * M:(kt + 1) * M],
                                  x_b[:, kt * P:(kt + 1) * P], transpose=True)
            with tc.tile_pool(name="psA", bufs=2, space="PSUM") as psA:
                for mt in range(MT):
                    pxa = psA.tile([P, H1], F32, tag="pxa")
                    for kt in range(F_IN // P):
                        nc.tensor.matmul(
                            pxa[:],
                            xT[:, kt * M + mt * P: kt * M + (mt + 1) * P],
                            W1b[:, kt * H1:(kt + 1) * H1],
                            start=(kt == 0), stop=(kt == F_IN // P - 1))
                    xw1_ev = ev.tile([P, H1], BF16, tag="xw1_ev")
                    nc.scalar.activation(xw1_ev[:], pxa[:], AF.Copy)
                    nc.sync.dma_start(xw1_bounce[mt * P:(mt + 1) * P, :], xw1_ev[:])
            nc.gpsimd.collective_compute(
                "AllGather", mybir.AluOpType.bypass, replica_groups=rg,
                ins=[xw1_bounce.opt()], outs=[xw1_all.opt()])
            # stationary tiles: xw1_sb[p, kt*H1+n] = xW1[kt*128+p, n]
            xw1_sb = pers.tile([P, KT * H1], BF16)
            nc.sync.dma_start(
                xw1_sb[:].rearrange("p (t n) -> p t n", n=H1),
                xw1_all.rearrange("(t p) n -> p t n", p=P))

            # ---------------- stage B: pass 1, hT = relu(xW1^T adjT) -------
            with tc.tile_pool(name="psB", bufs=1, space="PSUM") as psB:
                p1 = [psB.tile([P, 512], F32, tag=f"p1_{i}", name=f"p1_{i}")
                      for i in range(4)]
                for kt in range(KT):
                    R = mv.tile([P, M], BF16, tag="R1")
                    nc.sync.dma_start(R[:], adj_b[:, kt * P:(kt + 1) * P],
                                      transpose=True)
                    for nt in range(2):
                        for hf in range(2):
                            nc.tensor.matmul(
                                p1[nt * 2 + hf][:],
                                xw1_sb[:, kt * H1 + nt * P: kt * H1 + (nt + 1) * P],
                                R[:, hf * 512:(hf + 1) * 512],
                                start=(kt == 0), stop=(kt == KT - 1))
                for nt in range(2):
                    for hf in range(2):
                        h_ev = ev.tile([P, 512], BF16, tag="h_ev")
                        nc.scalar.activation(h_ev[:], p1[nt * 2 + hf][:], AF.Relu)
                        nc.sync.dma_start(
                            h_bounce[nt * P:(nt + 1) * P, hf * 512:(hf + 1) * 512],
                            h_ev[:])
            nc.gpsimd.collective_compute(
                "AllGather", mybir.AluOpType.bypass, replica_groups=rg,
                ins=[h_bounce.opt()], outs=[h_all.opt()])

            # ---------------- stage C: hWms_full (replicated) --------------
            # hT_sb[p, t*M+m] = h_all[t*128+p, m],  t = 2*core + dt
            hT_sb = pers.tile([P, (H1 * NC // P) * M], BF16)
            nc.sync.dma_start(
                hT_sb[:].rearrange("p (t m) -> p t m", m=M),
                h_all.rearrange("(t p) m -> p t m", p=P))
            hwms_sb = pers.tile([P, KT * 256], BF16)
            with tc.tile_pool(name="psC", bufs=2, space="PSUM") as psC:
                for kt in range(KT):
                    c, mloc = kt // MT, (kt % MT) * P
                    pc = psC.tile([P, 256], F32, tag="pc")
                    for dt in range(H1 // P):
                        nc.tensor.matmul(
                            pc[:],
                            hT_sb[:, (2 * c + dt) * M + mloc:
                                  (2 * c + dt) * M + mloc + P],
                            Wms[:, dt * 256:(dt + 1) * 256],
                            start=(dt == 0), stop=(dt == H1 // P - 1))
                    nc.vector.tensor_copy(hwms_sb[:, kt * 256:(kt + 1) * 256],
                                          pc[:])

            # ---------------- stage D: pass 2, zmT / lsT --------------------
            zmT = pers.tile([P, M], F32)
            lsT = pers.tile([P, M], F32)
            with tc.tile_pool(name="psD", bufs=1, space="PSUM") as psD:
                p2 = [psD.tile([P, 512], F32, tag=f"p2_{i}", name=f"p2_{i}")
                      for i in range(4)]
                for kt in range(KT):
                    R2 = mv.tile([P, M], BF16, tag="R2")
                    nc.sync.dma_start(R2[:], adj_b[:, kt * P:(kt + 1) * P],
                                      transpose=True)
                    for j in range(2):
                        for hf in range(2):
                            nc.tensor.matmul(
                                p2[j * 2 + hf][:],
                                hwms_sb[:, kt * 256 + j * P: kt * 256 + (j + 1) * P],
                                R2[:, hf * 512:(hf + 1) * 512],
                                start=(kt == 0), stop=(kt == KT - 1))
                for hf in range(2):
                    nc.vector.tensor_copy(zmT[:, hf * 512:(hf + 1) * 512],
                                          p2[0 * 2 + hf][:])
                    nc.vector.tensor_copy(lsT[:, hf * 512:(hf + 1) * 512],
                                          p2[1 * 2 + hf][:])

            # ---------------- stage E: z = zm + eps * exp(ls) --------------
            epsT = pers.tile([P, M], BF16)
            nc.sync.dma_start(epsT[:], eps_b[:, :], transpose=True)
            epsT_f = pers.tile([P, M], F32)
            nc.vector.tensor_copy(epsT_f[:], epsT[:])
            expT = pers.tile([P, M], F32)
            nc.scalar.activation(expT[:], lsT[:], AF.Exp)
            zT = pers.tile([P, M], F32)
            nc.vector.tensor_mul(zT[:], epsT_f[:], expT[:])
            nc.vector.tensor_add(zT[:], zmT[:], zT[:])
            zT_b = pers.tile([P, M], BF16)
            nc.vector.tensor_copy(zT_b[:], zT[:])
            nc.sync.dma_start(z_bounce[:, :], zT_b[:])
            nc.gpsimd.collective_compute(
                "AllGather", mybir.AluOpType.bypass, replica_groups=rg,
                ins=[z_bounce.opt()], outs=[z_all.opt()])

            # ---------------- stage F: decoder ------------------------------
            # z_sb[p, t*M+m] = z_all[t*128+p, m] = zT of core t
            z_sb = pers.tile([P, NC * M], BF16)
            nc.sync.dma_start(
                z_sb[:].rearrange("p (t m) -> p t m", m=M),
                z_all.rearrange("(t p) m -> p t m", p=P))
            with tc.tile_pool(name="psF", bufs=6, space="PSUM") as psF:
              for mt in range(MT):
                for nb in range(NC):
                    for nh in range(2):
                        pd = psF.tile([P, 512], F32, tag="pd")
                        nc.tensor.matmul(
                            pd[:],
                            zT_b[:, mt * P:(mt + 1) * P],
                            z_sb[:, nb * M + nh * 512: nb * M + (nh + 1) * 512],
                            start=True, stop=True)
                        o_ev = ev.tile([P, 512], F32, tag="o_ev")
                        nc.scalar.activation(o_ev[:], pd[:], AF.Sigmoid)
                        nc.sync.dma_start(
                            out_s[mt * P:(mt + 1) * P,
                                  nb * M + nh * 512: nb * M + (nh + 1) * 512],
                            o_ev[:])
    nc.compile()
    return nc


_CACHED = None


def kernel(x, adj, W1, Wm, Ws, eps):
    global _CACHED
    if _CACHED is None:
        _CACHED = build()
    nc = _CACHED
    in_maps = []
    for c in range(NC):
        r0 = c * M
        in_maps.append({
            "x_s": np.ascontiguousarray(x[r0:r0 + M]),
            "adj_s": np.ascontiguousarray(adj[r0:r0 + M]),
            "W1": np.ascontiguousarray(W1),
            "Wm": np.ascontiguousarray(Wm),
            "Ws": np.ascontiguousarray(Ws),
            "eps_s": np.ascontiguousarray(eps[r0:r0 + M]),
        })
    res = run_bass_kernel_spmd(nc, in_maps, core_ids=list(range(NC)))
    out = np.concatenate([res.results[c]["out_s"] for c in range(NC)], axis=0)
    return out.astype(np.float32, copy=False)


if __name__ == "__main__":
    rng = np.random.default_rng(0)
    out = kernel(
        rng.standard_normal((N, F_IN), dtype=np.float32),
        rng.random((N, N), dtype=np.float32),
        (rng.standard_normal((F_IN, H1)) / np.sqrt(F_IN)).astype(np.float32),
        (rng.standard_normal((H1, H2)) / np.sqrt(H1)).astype(np.float32),
        (rng.standard_normal((H1, H2)) / np.sqrt(H1)).astype(np.float32),
        rng.standard_normal((N, H2), dtype=np.float32),
    )
    print(out.shape, np.isnan(out).mean())


# revision 8
# speedup vs baseline: 1.0543x; 1.0543x over previous
"""GCN-VAE forward pass (GCNModelVAE) on 8 Trainium2 NeuronCores.

Row-shards the 8192 nodes across 8 cores (1024 rows each). All big matmuls
run in bf16 on the PE array with fp32 PSUM accumulation; the output is
saturation-dominated (the reference's exp(log_std) overflows), so bf16
operand precision is ample.

Dataflow per core (M = 1024 local nodes, P = 128 partitions):
  0. Cast x/eps/W fp32 -> bf16, then adj_s in 8 column chunks (SWDGE).
  A. xW1_s = x_s @ W1 via transposed tiles, AllGather -> xW1_full [8192,256].
  B. hT_s = relu(xW1_full^T @ adj_s^T) [256,1024]; adj^T tiles via DMA-xbar
     transpose loads alternating across the two HWDGE queues.
  C. hWms_s = h_s @ [Wm|Ws] [1024,256] locally, AllGather -> hWms_full.
  D. zmT/lsT = (hWms)^T @ adj_s^T [128,1024] each.
  E. zT = zmT + epsT * exp(lsT); AllGather zT -> z_all.
  F. out_s = sigmoid(zT_s^T @ zT_all) [1024,8192] row-block of the decoder.
"""

import numpy as np

import concourse.bacc as bacc
import concourse.mybir as mybir
import concourse.tile as tile
from concourse.bass_utils import run_bass_kernel_spmd

N = 8192
F_IN = 512
H1 = 256
H2 = 128
NC = 8
M = N // NC          # 1024 rows per core
P = 128
KT = N // P          # 64 k-tiles over the node dimension
MT = M // P          # 8 m-tiles per core
F32 = mybir.dt.float32
BF16 = mybir.dt.bfloat16
AF = mybir.ActivationFunctionType


def build(n_cores=NC):
    nc = bacc.Bacc("TRN2", target_bir_lowering=False, debug=False,
                   num_devices=n_cores)
    x_s = nc.dram_tensor("x_s", [M, F_IN], F32, kind="ExternalInput")
    adj_s = nc.dram_tensor("adj_s", [M, N], F32, kind="ExternalInput")
    W1 = nc.dram_tensor("W1", [F_IN, H1], F32, kind="ExternalInput")
    Wm = nc.dram_tensor("Wm", [H1, H2], F32, kind="ExternalInput")
    Ws = nc.dram_tensor("Ws", [H1, H2], F32, kind="ExternalInput")
    eps_s = nc.dram_tensor("eps_s", [M, H2], F32, kind="ExternalInput")
    out_s = nc.dram_tensor("out_s", [M, N], F32, kind="ExternalOutput")

    rg = [list(range(n_cores))]
    hwdge = [nc.sync, nc.scalar]

    with tile.TileContext(nc) as tc:
        with (
            tc.tile_pool(name="dram", bufs=1, space="DRAM") as dram,
            tc.tile_pool(name="pers", bufs=1) as pers,
            tc.tile_pool(name="mv", bufs=12) as mv,
            tc.tile_pool(name="ev", bufs=6) as ev,
        ):
            # ---------------- DRAM staging ----------------
            adj_bc = [dram.tile([M, M], BF16, name=f"adj_bc{c}")
                      for c in range(NC)]
            x_b = dram.tile([M, F_IN], BF16)
            eps_b = dram.tile([M, H2], BF16)
            xw1_bounce = dram.tile([M, H1], BF16)
            xw1_all = dram.tile([N, H1], BF16, addr_space="Shared")
            hwms_bounce = dram.tile([M, 2 * H2], BF16)
            hwms_all = dram.tile([N, 2 * H2], BF16, addr_space="Shared")
            z_bounce = dram.tile([H2, M], BF16)
            z_all = dram.tile([H2 * NC, M], BF16, addr_space="Shared")

            # ---------------- stage 0: small casts first ----------------
            nc.gpsimd.dma_start(x_b[:, :], x_s[:, :])
            nc.gpsimd.dma_start(eps_b[:, :], eps_s[:, :])
            W1b = pers.tile([P, (F_IN // P) * H1], BF16)
            nc.gpsimd.dma_start(
                W1b[:].rearrange("p (t n) -> p t n", n=H1),
                W1.rearrange("(t p) n -> p t n", p=P))
            Wms = pers.tile([P, (H1 // P) * (2 * H2)], BF16)
            for dt in range(H1 // P):
                nc.gpsimd.dma_start(Wms[:, dt * 256:dt * 256 + H2],
                                    Wm[dt * P:(dt + 1) * P, :])
                nc.gpsimd.dma_start(Wms[:, dt * 256 + H2:dt * 256 + 256],
                                    Ws[dt * P:(dt + 1) * P, :])
            # adj in 8 column chunks, in pass-1 consumption order
            for c in range(NC):
                nc.gpsimd.dma_start(adj_bc[c][:, :],
                                    adj_s[:, c * M:(c + 1) * M])

            # ---------------- stage A: xW1 shard + AllGather -------------
            xT = pers.tile([P, (F_IN // P) * M], BF16)
            for kt in range(F_IN // P):
                hwdge[kt % 2].dma_start(xT[:, kt * M:(kt + 1) * M],
                                        x_b[:, kt * P:(kt + 1) * P],
                                        transpose=True)
            with tc.tile_pool(name="psA", bufs=2, space="PSUM") as psA:
                for mt in range(MT):
                    pxa = psA.tile([P, H1], F32, tag="pxa")
                    for kt in range(F_IN // P):
                        nc.tensor.matmul(
                            pxa[:],
                            xT[:, kt * M + mt * P: kt * M + (mt + 1) * P],
                            W1b[:, kt * H1:(kt + 1) * H1],
                            start=(kt == 0), stop=(kt == F_IN // P - 1))
                    xw1_ev = ev.tile([P, H1], BF16, tag="xw1_ev")
                    nc.scalar.activation(xw1_ev[:], pxa[:], AF.Copy)
                    nc.sync.dma_start(xw1_bounce[mt * P:(mt + 1) * P, :],
                                      xw1_ev[:])
            nc.gpsimd.collective_compute(
                "AllGather", mybir.AluOpType.bypass, replica_groups=rg,
                ins=[xw1_bounce.opt()], outs=[xw1_all.opt()])
            xw1_sb = pers.tile([P, KT * H1], BF16)
            nc.sync.dma_start(
                xw1_sb[:].rearrange("p (t n) -> p t n", n=H1),
                xw1_all.rearrange("(t p) n -> p t n", p=P))

            # ---------------- stage B: pass 1, hT = relu(xW1^T adjT) -----
            hT_loc = pers.tile([P, 2 * M], BF16)
            with tc.tile_pool(name="psB", bufs=1, space="PSUM") as psB:
                p1 = [psB.tile([P, 512], F32, tag=f"p1_{i}", name=f"p1_{i}")
                      for i in range(4)]
                for kt in range(KT):
                    R = mv.tile([P, M], BF16, tag="R", name="R")
                    hwdge[kt % 2].dma_start(
                        R[:], adj_bc[kt // MT][:, (kt % MT) * P:
                                               (kt % MT + 1) * P],
                        transpose=True)
                    for nt in range(2):
                        for hf in range(2):
                            nc.tensor.matmul(
                                p1[nt * 2 + hf][:],
                                xw1_sb[:, kt * H1 + nt * P:
                                       kt * H1 + (nt + 1) * P],
                                R[:, hf * 512:(hf + 1) * 512],
                                start=(kt == 0), stop=(kt == KT - 1))
                for nt in range(2):
                    for hf in range(2):
                        nc.scalar.activation(
                            hT_loc[:, nt * M + hf * 512:
                                   nt * M + (hf + 1) * 512],
                            p1[nt * 2 + hf][:], AF.Relu)

            # ---------------- stage C: local hWms shard + AllGather -------
            with tc.tile_pool(name="psC", bufs=2, space="PSUM") as psC:
                for mt in range(MT):
                    pc = psC.tile([P, 2 * H2], F32, tag="pc")
                    for dt in range(H1 // P):
                        nc.tensor.matmul(
                            pc[:],
                            hT_loc[:, dt * M + mt * P: dt * M + (mt + 1) * P],
                            Wms[:, dt * 256:(dt + 1) * 256],
                            start=(dt == 0), stop=(dt == H1 // P - 1))
                    hw_ev = ev.tile([P, 2 * H2], BF16, tag="hw_ev")
                    nc.vector.tensor_copy(hw_ev[:], pc[:])
                    nc.sync.dma_start(hwms_bounce[mt * P:(mt + 1) * P, :],
                                      hw_ev[:])
            nc.gpsimd.collective_compute(
                "AllGather", mybir.AluOpType.bypass, replica_groups=rg,
                ins=[hwms_bounce.opt()], outs=[hwms_all.opt()])
            hwms_sb = pers.tile([P, KT * 256], BF16)
            nc.sync.dma_start(
                hwms_sb[:].rearrange("p (t n) -> p t n", n=256),
                hwms_all.rearrange("(t p) n -> p t n", p=P))

            # ---------------- stage D: pass 2, zmT / lsT ------------------
            zmT = pers.tile([P, M], F32)
            lsT = pers.tile([P, M], F32)
            with tc.tile_pool(name="psD", bufs=1, space="PSUM") as psD:
                p2 = [psD.tile([P, 512], F32, tag=f"p2_{i}", name=f"p2_{i}")
                      for i in range(4)]
                for kt in range(KT):
                    R2 = mv.tile([P, M], BF16, tag="R", name="R2")
                    hwdge[kt % 2].dma_start(
                        R2[:], adj_bc[kt // MT][:, (kt % MT) * P:
                                                (kt % MT + 1) * P],
                        transpose=True)
                    for j in range(2):
                        for hf in range(2):
                            nc.tensor.matmul(
                                p2[j * 2 + hf][:],
                                hwms_sb[:, kt * 256 + j * P:
                                        kt * 256 + (j + 1) * P],
                                R2[:, hf * 512:(hf + 1) * 512],
                                start=(kt == 0), stop=(kt == KT - 1))
                for hf in range(2):
                    nc.vector.tensor_copy(zmT[:, hf * 512:(hf + 1) * 512],
                                          p2[0 * 2 + hf][:])
                    nc.vector.tensor_copy(lsT[:, hf * 512:(hf + 1) * 512],
                                          p2[1 * 2 + hf][:])

            # ---------------- stage E: z = zm + eps * exp(ls) -------------
            epsT = pers.tile([P, M], BF16)
            nc.scalar.dma_start(epsT[:], eps_b[:, :], transpose=True)
            epsT_f = pers.tile([P, M], F32)
            nc.vector.tensor_copy(epsT_f[:], epsT[:])
            expT = pers.tile([P, M], F32)
            nc.scalar.activation(expT[:], lsT[:], AF.Exp)
            zT = pers.tile([P, M], F32)
            nc.vector.tensor_mul(zT[:], epsT_f[:], expT[:])
            nc.vector.tensor_add(zT[:], zmT[:], zT[:])
            zT_b = pers.tile([P, M], BF16)
            nc.vector.tensor_copy(zT_b[:], zT[:])
            nc.sync.dma_start(z_bounce[:, :], zT_b[:])
            nc.gpsimd.collective_compute(
                "AllGather", mybir.AluOpType.bypass, replica_groups=rg,
                ins=[z_bounce.opt()], outs=[z_all.opt()])

            # ---------------- stage F: decoder ----------------------------
            z_sb = pers.tile([P, NC * M], BF16)
            nc.sync.dma_start(
                z_sb[:].rearrange("p (t m) -> p t m", m=M),
                z_all.rearrange("(t p) m -> p t m", p=P))
            with tc.tile_pool(name="psF", bufs=6, space="PSUM") as psF:
                i = 0
                for mt in range(MT):
                    for nb in range(NC):
                        for nh in range(2):
                            pd = psF.tile([P, 512], F32, tag="pd")
                            nc.tensor.matmul(
                                pd[:],
                                zT_b[:, mt * P:(mt + 1) * P],
                                z_sb[:, nb * M + nh * 512:
                                     nb * M + (nh + 1) * 512],
                                start=True, stop=True)
                            o_ev = ev.tile([P, 512], F32, tag="o_ev")
                            nc.scalar.activation(o_ev[:], pd[:], AF.Sigmoid)
                            eng = nc.sync if i % 2 == 0 else nc.gpsimd
                            eng.dma_start(
                                out_s[mt * P:(mt + 1) * P,
                                      nb * M + nh * 512:
                                      nb * M + (nh + 1) * 512],
                                o_ev[:])
                            i += 1
    nc.compile()
    return nc


_CACHED = None


def kernel(x, adj, W1, Wm, Ws, eps):
    global _CACHED
    if _CACHED is None:
        _CACHED = build()
    nc = _CACHED
    in_maps = []
    for c in range(NC):
        r0 = c * M
        in_maps.append({
            "x_s": np.ascontiguousarray(x[r0:r0 + M]),
            "adj_s": np.ascontiguousarray(adj[r0:r0 + M]),
            "W1": np.ascontiguousarray(W1),
            "Wm": np.ascontiguousarray(Wm),
            "Ws": np.ascontiguousarray(Ws),
            "eps_s": np.ascontiguousarray(eps[r0:r0 + M]),
        })
    res = run_bass_kernel_spmd(nc, in_maps, core_ids=list(range(NC)))
    out = np.concatenate([res.results[c]["out_s"] for c in range(NC)], axis=0)
    return out.astype(np.float32, copy=False)


if __name__ == "__main__":
    rng = np.random.default_rng(0)
    out = kernel(
        rng.standard_normal((N, F_IN), dtype=np.float32),
        rng.random((N, N), dtype=np.float32),
        (rng.standard_normal((F_IN, H1)) / np.sqrt(F_IN)).astype(np.float32),
        (rng.standard_normal((H1, H2)) / np.sqrt(H1)).astype(np.float32),
        (rng.standard_normal((H1, H2)) / np.sqrt(H1)).astype(np.float32),
        rng.standard_normal((N, H2), dtype=np.float32),
    )
    print(out.shape, np.isnan(out).mean())


# revision 9
# speedup vs baseline: 1.0580x; 1.0035x over previous
"""GCN-VAE forward pass (GCNModelVAE) on 8 Trainium2 NeuronCores.

Row-shards the 8192 nodes across 8 cores (1024 rows each). All big matmuls
run in bf16 on the PE array with fp32 PSUM accumulation; the output is
saturation-dominated (the reference's exp(log_std) overflows), so bf16
operand precision is ample.

Dataflow per core (M = 1024 local nodes, P = 128 partitions):
  0. Cast x/eps/W fp32 -> bf16, then adj_s in 8 column chunks (SWDGE).
  A. xW1_s = x_s @ W1 via transposed tiles, AllGather -> xW1_full [8192,256].
  B. hT_s = relu(xW1_full^T @ adj_s^T) [256,1024]; adj^T tiles via DMA-xbar
     transpose loads alternating across the two HWDGE queues.
  C. hWms_s = h_s @ [Wm|Ws] [1024,256] locally, AllGather -> hWms_full.
  D. zmT/lsT = (hWms)^T @ adj_s^T [128,1024] each.
  E. zT = zmT + epsT * exp(lsT); AllGather zT -> z_all.
  F. out_s = sigmoid(zT_s^T @ zT_all) [1024,8192] row-block of the decoder.
"""

import numpy as np

import concourse.bacc as bacc
import concourse.mybir as mybir
import concourse.tile as tile
from concourse.bass_utils import run_bass_kernel_spmd

N = 8192
F_IN = 512
H1 = 256
H2 = 128
NC = 8
M = N // NC          # 1024 rows per core
P = 128
KT = N // P          # 64 k-tiles over the node dimension
MT = M // P          # 8 m-tiles per core
F32 = mybir.dt.float32
BF16 = mybir.dt.bfloat16
AF = mybir.ActivationFunctionType


def build(n_cores=NC):
    nc = bacc.Bacc("TRN2", target_bir_lowering=False, debug=False,
                   num_devices=n_cores)
    x_s = nc.dram_tensor("x_s", [M, F_IN], F32, kind="ExternalInput")
    adj_s = nc.dram_tensor("adj_s", [M, N], F32, kind="ExternalInput")
    W1 = nc.dram_tensor("W1", [F_IN, H1], F32, kind="ExternalInput")
    Wm = nc.dram_tensor("Wm", [H1, H2], F32, kind="ExternalInput")
    Ws = nc.dram_tensor("Ws", [H1, H2], F32, kind="ExternalInput")
    eps_s = nc.dram_tensor("eps_s", [M, H2], F32, kind="ExternalInput")
    out_s = nc.dram_tensor("out_s", [M, N], F32, kind="ExternalOutput")

    rg = [list(range(n_cores))]
    hwdge = [nc.sync, nc.scalar]

    with tile.TileContext(nc) as tc:
        with (
            tc.tile_pool(name="dram", bufs=1, space="DRAM") as dram,
            tc.tile_pool(name="pers", bufs=1) as pers,
            tc.tile_pool(name="mv", bufs=12) as mv,
            tc.tile_pool(name="ev", bufs=6) as ev,
        ):
            # ---------------- DRAM staging ----------------
            adj_bc = [dram.tile([M, M], BF16, name=f"adj_bc{c}")
                      for c in range(NC)]
            x_b = dram.tile([M, F_IN], BF16)
            eps_b = dram.tile([M, H2], BF16)
            xw1_bounce = dram.tile([M, H1], BF16)
            xw1_all = dram.tile([N, H1], BF16, addr_space="Shared")
            hwms_bounce = dram.tile([M, 2 * H2], BF16)
            hwms_all = dram.tile([N, 2 * H2], BF16, addr_space="Shared")
            z_bounce = dram.tile([H2, M], BF16)
            z_all = dram.tile([H2 * NC, M], BF16, addr_space="Shared")

            # ---------------- stage 0: small casts first ----------------
            nc.gpsimd.dma_start(x_b[:, :], x_s[:, :])
            nc.gpsimd.dma_start(eps_b[:, :], eps_s[:, :])
            W1b = pers.tile([P, (F_IN // P) * H1], BF16)
            nc.gpsimd.dma_start(
                W1b[:].rearrange("p (t n) -> p t n", n=H1),
                W1.rearrange("(t p) n -> p t n", p=P))
            Wms = pers.tile([P, (H1 // P) * (2 * H2)], BF16)
            for dt in range(H1 // P):
                nc.gpsimd.dma_start(Wms[:, dt * 256:dt * 256 + H2],
                                    Wm[dt * P:(dt + 1) * P, :])
                nc.gpsimd.dma_start(Wms[:, dt * 256 + H2:dt * 256 + 256],
                                    Ws[dt * P:(dt + 1) * P, :])
            # adj in 8 column chunks, in pass-1 consumption order
            for c in range(NC):
                nc.gpsimd.dma_start(adj_bc[c][:, :],
                                    adj_s[:, c * M:(c + 1) * M])

            # ---------------- stage A: xW1 shard + AllGather -------------
            xT = pers.tile([P, (F_IN // P) * M], BF16)
            for kt in range(F_IN // P):
                hwdge[kt % 2].dma_start(xT[:, kt * M:(kt + 1) * M],
                                        x_b[:, kt * P:(kt + 1) * P],
                                        transpose=True)
            with tc.tile_pool(name="psA", bufs=2, space="PSUM") as psA:
                for mt in range(MT):
                    pxa = psA.tile([P, H1], F32, tag="pxa")
                    for kt in range(F_IN // P):
                        nc.tensor.matmul(
                            pxa[:],
                            xT[:, kt * M + mt * P: kt * M + (mt + 1) * P],
                            W1b[:, kt * H1:(kt + 1) * H1],
                            start=(kt == 0), stop=(kt == F_IN // P - 1))
                    xw1_ev = ev.tile([P, H1], BF16, tag="xw1_ev")
                    nc.vector.tensor_copy(xw1_ev[:], pxa[:])
                    nc.gpsimd.dma_start(xw1_bounce[mt * P:(mt + 1) * P, :],
                                        xw1_ev[:])
            nc.gpsimd.collective_compute(
                "AllGather", mybir.AluOpType.bypass, replica_groups=rg,
                ins=[xw1_bounce.opt()], outs=[xw1_all.opt()])
            xw1_sb = pers.tile([P, KT * H1], BF16)
            nc.gpsimd.dma_start(
                xw1_sb[:].rearrange("p (t n) -> p t n", n=H1),
                xw1_all.rearrange("(t p) n -> p t n", p=P))

            # ---------------- stage B: pass 1, hT = relu(xW1^T adjT) -----
            hT_loc = pers.tile([P, 2 * M], BF16)
            with tc.tile_pool(name="psB", bufs=1, space="PSUM") as psB:
                p1 = [psB.tile([P, 512], F32, tag=f"p1_{i}", name=f"p1_{i}")
                      for i in range(4)]
                for kt in range(KT):
                    R = mv.tile([P, M], BF16, tag="R", name="R")
                    hwdge[kt % 2].dma_start(
                        R[:], adj_bc[kt // MT][:, (kt % MT) * P:
                                               (kt % MT + 1) * P],
                        transpose=True)
                    for nt in range(2):
                        for hf in range(2):
                            nc.tensor.matmul(
                                p1[nt * 2 + hf][:],
                                xw1_sb[:, kt * H1 + nt * P:
                                       kt * H1 + (nt + 1) * P],
                                R[:, hf * 512:(hf + 1) * 512],
                                start=(kt == 0), stop=(kt == KT - 1))
                for nt in range(2):
                    for hf in range(2):
                        nc.scalar.activation(
                            hT_loc[:, nt * M + hf * 512:
                                   nt * M + (hf + 1) * 512],
                            p1[nt * 2 + hf][:], AF.Relu)

            # ---------------- stage C: local hWms shard + AllGather -------
            with tc.tile_pool(name="psC", bufs=2, space="PSUM") as psC:
                for mt in range(MT):
                    pc = psC.tile([P, 2 * H2], F32, tag="pc")
                    for dt in range(H1 // P):
                        nc.tensor.matmul(
                            pc[:],
                            hT_loc[:, dt * M + mt * P: dt * M + (mt + 1) * P],
                            Wms[:, dt * 256:(dt + 1) * 256],
                            start=(dt == 0), stop=(dt == H1 // P - 1))
                    hw_ev = ev.tile([P, 2 * H2], BF16, tag="hw_ev")
                    nc.vector.tensor_copy(hw_ev[:], pc[:])
                    nc.gpsimd.dma_start(hwms_bounce[mt * P:(mt + 1) * P, :],
                                        hw_ev[:])
            nc.gpsimd.collective_compute(
                "AllGather", mybir.AluOpType.bypass, replica_groups=rg,
                ins=[hwms_bounce.opt()], outs=[hwms_all.opt()])
            hwms_sb = pers.tile([P, KT * 256], BF16)
            nc.gpsimd.dma_start(
                hwms_sb[:].rearrange("p (t n) -> p t n", n=256),
                hwms_all.rearrange("(t p) n -> p t n", p=P))

            # ---------------- stage D: pass 2, zmT / lsT ------------------
            zmT = pers.tile([P, M], F32)
            lsT = pers.tile([P, M], F32)
            with tc.tile_pool(name="psD", bufs=1, space="PSUM") as psD:
                p2 = [psD.tile([P, 512], F32, tag=f"p2_{i}", name=f"p2_{i}")
                      for i in range(4)]
                for kt in range(KT):
                    R2 = mv.tile([P, M], BF16, tag="R", name="R2")
                    hwdge[kt % 2].dma_start(
                        R2[:], adj_bc[kt // MT][:, (kt % MT) * P:
                                                (kt % MT + 1) * P],
                        transpose=True)
                    for j in range(2):
                        for hf in range(2):
                            nc.tensor.matmul(
                                p2[j * 2 + hf][:],
                                hwms_sb[:, kt * 256 + j * P:
                                        kt * 256 + (j + 1) * P],
                                R2[:, hf * 512:(hf + 1) * 512],
                                start=(kt == 0), stop=(kt == KT - 1))
                for hf in range(2):
                    nc.vector.tensor_copy(zmT[:, hf * 512:(hf + 1) * 512],
                                          p2[0 * 2 + hf][:])
                    nc.vector.tensor_copy(lsT[:, hf * 512:(hf + 1) * 512],
                                          p2[1 * 2 + hf][:])

            # ---------------- stage E: z = zm + eps * exp(ls) -------------
            epsT = pers.tile([P, M], BF16)
            nc.scalar.dma_start(epsT[:], eps_b[:, :], transpose=True)
            epsT_f = pers.tile([P, M], F32)
            nc.vector.tensor_copy(epsT_f[:], epsT[:])
            expT = pers.tile([P, M], F32)
            nc.scalar.activation(expT[:], lsT[:], AF.Exp)
            zT = pers.tile([P, M], F32)
            nc.vector.tensor_mul(zT[:], epsT_f[:], expT[:])
            nc.vector.tensor_add(zT[:], zmT[:], zT[:])
            zT_b = pers.tile([P, M], BF16)
            nc.vector.tensor_copy(zT_b[:], zT[:])
            nc.gpsimd.dma_start(z_bounce[:, :], zT_b[:])
            nc.gpsimd.collective_compute(
                "AllGather", mybir.AluOpType.bypass, replica_groups=rg,
                ins=[z_bounce.opt()], outs=[z_all.opt()])

            # ---------------- stage F: decoder ----------------------------
            z_sb = pers.tile([P, NC * M], BF16)
            for t in range(NC):
                nc.gpsimd.dma_start(z_sb[:, t * M:(t + 1) * M],
                                    z_all[t * P:(t + 1) * P, :])
            with tc.tile_pool(name="psF", bufs=6, space="PSUM") as psF:
                i = 0
                for mt in range(MT):
                    for nb in range(NC):
                        for nh in range(2):
                            pd = psF.tile([P, 512], F32, tag="pd")
                            nc.tensor.matmul(
                                pd[:],
                                zT_b[:, mt * P:(mt + 1) * P],
                                z_sb[:, nb * M + nh * 512:
                                     nb * M + (nh + 1) * 512],
                                start=True, stop=True)
                            o_ev = ev.tile([P, 512], F32, tag="o_ev")
                            nc.scalar.activation(o_ev[:], pd[:], AF.Sigmoid)
                            nc.gpsimd.dma_start(
                                out_s[mt * P:(mt + 1) * P,
                                      nb * M + nh * 512:
                                      nb * M + (nh + 1) * 512],
                                o_ev[:])
                            i += 1
    nc.compile()
    return nc


_CACHED = None


def kernel(x, adj, W1, Wm, Ws, eps):
    global _CACHED
    if _CACHED is None:
        _CACHED = build()
    nc = _CACHED
    in_maps = []
    for c in range(NC):
        r0 = c * M
        in_maps.append({
            "x_s": np.ascontiguousarray(x[r0:r0 + M]),
            "adj_s": np.ascontiguousarray(adj[r0:r0 + M]),
            "W1": np.ascontiguousarray(W1),
            "Wm": np.ascontiguousarray(Wm),
            "Ws": np.ascontiguousarray(Ws),
            "eps_s": np.ascontiguousarray(eps[r0:r0 + M]),
        })
    res = run_bass_kernel_spmd(nc, in_maps, core_ids=list(range(NC)))
    out = np.concatenate([res.results[c]["out_s"] for c in range(NC)], axis=0)
    return out.astype(np.float32, copy=False)


if __name__ == "__main__":
    rng = np.random.default_rng(0)
    out = kernel(
        rng.standard_normal((N, F_IN), dtype=np.float32),
        rng.random((N, N), dtype=np.float32),
        (rng.standard_normal((F_IN, H1)) / np.sqrt(F_IN)).astype(np.float32),
        (rng.standard_normal((H1, H2)) / np.sqrt(H1)).astype(np.float32),
        (rng.standard_normal((H1, H2)) / np.sqrt(H1)).astype(np.float32),
        rng.standard_normal((N, H2), dtype=np.float32),
    )
    print(out.shape, np.isnan(out).mean())
